# revision 6
# baseline (speedup 1.0000x reference)
"""Segment-mean (ConfidenceLayer) Trainium2 kernel, v3.

out[s, b, :] = mean over pixels p with (slic[b,p] - 1 == s) of img[b,p,:]

One batch element per NeuronCore (8 cores).  v3 architecture:

- Masks (one-hot rows scaled by a known per-producer constant alpha) are
  produced per pixel-column by two engines:
    * DVE: a custom paged DVE op emits, for 8 pixel-columns at a time,
      int16 tiles [128, 8, 128] where halfword n of page j is
      60 (0x003C) / 15360 (0x3C00) / 0 -- i.e. the fp8 byte stream of the
      tile is exactly the 256-segment one-hot with value fp8(0x3C)=1.5.
      ~142ns per pixel-column (vs 201ns for stock tensor_scalar).
    * ACT: Derivative_Erf bump with fp8 output [128, 256]; peak value
      quantizes to fp8 1.125 (0x39), tails exactly 0.  ~399ns/column.
- The matmul runs in fp8 DoubleRow mode, one MM per PAIR of pixel
  columns: lhsT [128, 2, 80] packs (pair slot) x (33 hi | 33 res | pad)
  and rhs is the bitcast fp8 view [128, 2, 256] of two mask tiles.
  ~109ns per pair -> tensor engine ~115us, no longer a bottleneck.
- Precision: img is sent as hi' = fp8(x/alpha) plus residual
  res' = fp8(16*(x - alpha*hi')/alpha); the MM accumulates
  alpha*(hi' + res'/16) = x with second-order fp8 error (~0.1%).
  The ones channel (counts) goes through the same machinery.
- Epilogue: merge 2 PSUM parity chains, sums = hi_rows + res_rows/16,
  transpose, divide by counts, DMA out [256, 32] f32.
"""

import numpy as np
import ml_dtypes

import concourse.bacc as bacc
import concourse.bass as bass
import concourse.tile as tile
from concourse import mybir
from concourse import dve_ops
from concourse.dve_ops import DveOp, OPS, _SUB_OPCODE_FOR_NAME, _CUSTOM_DVE_ROW_BASE
from concourse.dve_spec import (
    Spec, Src0, Src1, C0, C1, Idx, PageIdx, eq, lower, _has_src1,
)
from concourse.dve_uop import DveOpSpec
from concourse.bass_utils import run_bass_kernel_spmd

P = 128          # SBUF partitions
C = 32           # channels
S = 256          # segments
NH = 128         # halfwords per packed mask tile (2 segments each)
NQ = 64          # int32 words per packed mask tile (4 segments each)
MDR = 80         # DoubleRow lhsT column count (33 hi + 33 res + 14 pad)
B = 8
HW = 512 * 512
NPP = HW // P    # pixel-columns per partition (2048)
N_CORES = 8

ALPHA_DVE = 2.0     # fp8 value of byte 0x40
ALPHA_ACT = 1.125   # fp8 value of byte 0x39 (DErf peak, measured)
RES_SCALE = 16.0

# period-40 producer assignment: columns 0..33 -> DVE, 34..39 -> ACT
PERIOD = 40
N_DVE_COLS = 34

LAST_EXEC_NS = None

F16 = mybir.dt.float16
F32 = mybir.dt.float32
F8 = mybir.dt.float8e4
I16 = mybir.dt.int16
I32 = mybir.dt.int32

_ONEHOT_OP = None


def _col_kind(n):
    """True if pixel-column n is DVE-produced."""
    r = n % PERIOD
    return r >= (PERIOD - N_DVE_COLS) or n >= (NPP // PERIOD) * PERIOD


def _onehot_ref(in0, in1, s0, s1, imm2):
    Sd, Nd = in0.shape[1], in0.shape[2]
    idx = np.arange(Sd * Nd, dtype=np.float32).reshape(Sd, Nd)
    pg = s0 + np.arange(Sd, dtype=np.float32)[:, None] * s1
    return ((in0 + pg[None]) == idx[None]).astype(np.float32) * in1


def _register_op():
    global _ONEHOT_OP
    if _ONEHOT_OP is not None:
        return _ONEHOT_OP
    if "ONEHOT_PK" in _SUB_OPCODE_FOR_NAME:
        _ONEHOT_OP = next(o for o in OPS if o.name == "ONEHOT_PK")
        return _ONEHOT_OP
    _pg = PageIdx(C0, C1)
    spec = Spec(body=eq(Src0 + _pg, Idx) * Src1, reference=_onehot_ref)
    from concourse.bass import dve_ver_for

    ver = dve_ver_for("TRN2")
    row = _CUSTOM_DVE_ROW_BASE + len(OPS)
    assert row < 0x20, row
    uops = lower(spec, ver=ver)
    tmp = DveOpSpec(name="ONEHOT_PK", opcode=row, uops=uops,
                    rd1_en=_has_src1(spec))
    op = DveOp("ONEHOT_PK", spec, subdim=True,
               uops_sha={ver: tmp.sha(ver)})
    OPS.append(op)
    _SUB_OPCODE_FOR_NAME["ONEHOT_PK"] = row
    _ONEHOT_OP = op
    return op


def build_kernel():
    op = _register_op()
    nc = bacc.Bacc(None, target_bir_lowering=False)
    # img8: per pair of pixel-columns, [2 slots, 66 cols] fp8 (hi|res)
    img_d = nc.declare_dram_parameter(
        "img8", [P, NPP // 2, 2, MDR], F8, isOutput=False)
    slic_d = nc.declare_dram_parameter("slic", [P * NPP], mybir.dt.int32,
                                       isOutput=False)
    iota_d = nc.declare_dram_parameter("iota", [P, S], F16, isOutput=False)
    ident_d = nc.declare_dram_parameter("ident", [33, 33], F32, isOutput=False)
    out_d = nc.declare_dram_parameter("out", [S, C], F32, isOutput=True)

    slic_v = slic_d.rearrange("(p n) -> p n", p=P)  # [128, NPP]

    n_pairs = NPP // 2

    with tile.TileContext(nc) as tc:
        with (
            tc.tile_pool(name="const", bufs=1) as const_pool,
            tc.tile_pool(name="seg", bufs=1) as seg_pool,
            tc.tile_pool(name="img", bufs=4) as img_pool,
            tc.tile_pool(name="ohd", bufs=4) as ohd_pool,
            tc.tile_pool(name="oha", bufs=8) as oha_pool,
            tc.tile_pool(name="psum", bufs=2, space=bass.MemorySpace.PSUM) as psum_pool,
            tc.tile_pool(name="epi", bufs=1) as epi_pool,
            tc.tile_pool(name="epips", bufs=2, space=bass.MemorySpace.PSUM) as epips_pool,
        ):
            # ---- slic load + derived tiles, chunked ----
            slic_chunks = [(0, 64), (64, 64), (128, 128), (256, 256),
                           (512, 512), (1024, 512), (1536, 512)]
            n_slic_chunks = len(slic_chunks)
            slic_f = seg_pool.tile([P, NPP], F32, name="slic_f")
            val_i = seg_pool.tile([P, NPP], I32, name="val_i")
            bias_f = seg_pool.tile([P, NPP], F32, name="bias_f")
            q_i = seg_pool.tile([P, NPP], I16, name="q_i")
            t_f = seg_pool.tile([P, NPP], F32, name="t_f")
            e_i = seg_pool.tile([P, NPP], I32, name="e_i")
            c64 = seg_pool.tile([P, 1], I32, name="c64")
            nc.vector.memset(c64[:], 64)

            slic_raw = seg_pool.tile([P, 64], I32, name="slic_raw")

            def emit_slic_chunk(ci):
                c0, sz = slic_chunks[ci]
                sl = slice(c0, c0 + sz)
                if ci == 0:
                    # sync-queue raw load + DVE cast: starts earlier than SWDGE
                    nc.sync.dma_start(slic_raw[:, 0:sz], slic_v[:, sl])
                    nc.vector.tensor_copy(slic_f[:, sl], slic_raw[:, 0:sz])
                else:
                    # int32 -> f32 cast during SWDGE DMA
                    nc.gpsimd.dma_start(slic_f[:, sl], slic_v[:, sl])
                # q = floor((slic-1)/4) via round(0.25*slic - 0.625) to int16
                nc.vector.tensor_scalar(
                    q_i[:, sl], slic_f[:, sl], 0.25, -0.625,
                    mybir.AluOpType.mult, mybir.AluOpType.add)
                # r = slic - (4q+1);  e = 8r;  val = 64 << e
                nc.vector.tensor_scalar(
                    t_f[:, sl], q_i[:, sl], 4.0, 1.0,
                    mybir.AluOpType.mult, mybir.AluOpType.add)
                nc.vector.tensor_tensor(
                    t_f[:, sl], slic_f[:, sl], t_f[:, sl],
                    mybir.AluOpType.subtract)
                nc.vector.tensor_scalar(
                    e_i[:, sl], t_f[:, sl], 8.0, None,
                    mybir.AluOpType.mult)
                nc.vector.tensor_tensor(
                    val_i[:, sl], c64[:].broadcast_to([P, sz]), e_i[:, sl],
                    mybir.AluOpType.logical_shift_left)
                # ACT bias = -8*slic
                nc.vector.tensor_scalar(
                    bias_f[:, sl], slic_f[:, sl], -8.0, None,
                    mybir.AluOpType.mult)

            iota_t = const_pool.tile([P, S], F16)
            nc.sync.dma_start(iota_t[:], iota_d[:])
            ident_t = const_pool.tile([33, 33], F32)
            nc.sync.dma_start(ident_t[:], ident_d[:])

            emit_slic_chunk(0)
            emit_slic_chunk(1)

            # ---- accumulators: 2 parity chains [80, 256] ----
            acc = [psum_pool.tile([MDR, S], F32, name=f"acc{g}")
                   for g in range(2)]

            # ---- img pair-chunk list (small leading chunks) ----
            chunk_list = []
            lead = [16, 16, 32, 64]
            pos = 0
            for sz in lead:
                chunk_list.append((pos, sz))
                pos += sz
            while pos < n_pairs:
                sz = min(128, n_pairs - pos)
                chunk_list.append((pos, sz))
                pos += sz
            assert pos == n_pairs

            next_slic = 2

            # mask-tile generators, emitted lazily in pair order
            pair_rhs = {}  # pair index -> (rhs AP)

            def emit_masks_upto(t):
                """Ensure rhs for pair t exists; generates producer ops."""
                while emit_masks_upto.next_pair <= t:
                    tp = emit_masks_upto.next_pair
                    n0 = 2 * tp
                    if _col_kind(n0):
                        # DVE block: widest of 16/8/4/2 consecutive columns
                        w = 2
                        for cand in (32, 16, 8, 4):
                            if n0 + cand <= NPP and all(
                                _col_kind(n0 + i) for i in range(cand)
                            ):
                                w = cand
                                break
                        oh = ohd_pool.tile([P, w, NQ], I32)
                        nc.vector._custom_dve(
                            op, out=oh[:],
                            in0=q_i[:, n0:n0 + w][:, :, None]
                                .broadcast_to([P, w, NQ]),
                            in1=val_i[:, n0:n0 + w][:, :, None]
                                .broadcast_to([P, w, NQ]),
                            s0=0.0, s1=float(NQ),
                        )
                        oh8 = oh[:].bitcast(F8)  # [P, w, 256]
                        for k in range(w // 2):
                            pair_rhs[tp + k] = oh8[:, 2 * k:2 * k + 2, :]
                        emit_masks_upto.next_pair += w // 2
                    else:
                        # ACT pair tile
                        oh = oha_pool.tile([P, 2, S], F8)
                        for i in range(2):
                            n = n0 + i
                            nc.scalar.activation(
                                oh[:, i, :], iota_t[:],
                                mybir.ActivationFunctionType.Derivative_Erf,
                                bias=bias_f[:, n:n + 1], scale=8.0,
                            )
                        pair_rhs[tp] = oh[:]
                        emit_masks_upto.next_pair += 1

            emit_masks_upto.next_pair = 0

            for ki, (pbase, psz) in enumerate(chunk_list):
                img_t = img_pool.tile([P, psz, 2, MDR], F8)
                nc.gpsimd.dma_start(
                    img_t[:], img_d[:, pbase:pbase + psz, :, :],
                )
                for _ in range(2):
                    if next_slic < n_slic_chunks:
                        emit_slic_chunk(next_slic)
                        next_slic += 1

                for k in range(psz):
                    t = pbase + k
                    emit_masks_upto(t)
                    g = t % 2
                    nc.tensor.matmul(
                        acc[g][:], img_t[:, k, :, :], pair_rhs.pop(t),
                        start=(t < 2), stop=(t >= n_pairs - 2),
                        perf_mode=mybir.MatmulPerfMode.DoubleRow,
                        skip_group_check=True,
                    )

            # ---- epilogue: split by 128-col half to pipeline the serial
            # chain; the two DMA shifts go to different queues ----
            for hblk in range(S // P):
                blk = slice(hblk * P, (hblk + 1) * P)
                tot = epi_pool.tile([66, P], F32, name=f"tot{hblk}")
                nc.vector.tensor_copy(tot[:], acc[0][0:66, blk])
                nc.vector.tensor_tensor(
                    tot[:], tot[:], acc[1][0:66, blk], mybir.AluOpType.add)
                # partition slices must start 32-aligned: DMA rows 33:66 to 0
                res33 = epi_pool.tile([33, P], F32, name=f"res33_{hblk}")
                if hblk == 0:
                    nc.sync.dma_start(res33[:], tot[33:66, :])
                else:
                    nc.gpsimd.dma_start(res33[:], tot[33:66, :])
                sums = epi_pool.tile([33, P], F32, name=f"sums{hblk}")
                nc.vector.tensor_scalar(
                    sums[:], res33[:], 1.0 / RES_SCALE, None,
                    mybir.AluOpType.mult)
                nc.vector.tensor_tensor(
                    sums[:], sums[:], tot[0:33, :], mybir.AluOpType.add)
                tp = epips_pool.tile([P, 33], F32)
                nc.tensor.transpose(tp[:], sums[:], ident_t[:])
                tp_sb = epi_pool.tile([P, 33], F32, name=f"tpsb{hblk}")
                nc.vector.tensor_copy(tp_sb[:], tp[:])
                recip = epi_pool.tile([P, 1], F32, name=f"recip{hblk}")
                nc.vector.reciprocal(recip[:], tp_sb[:, 32:33])
                res = epi_pool.tile([P, C], F32, name=f"rs{hblk}")
                nc.vector.tensor_scalar(
                    res[:], tp_sb[:, 0:C], recip[:, 0:1], None,
                    mybir.AluOpType.mult,
                )
                nc.sync.dma_start(out_d[blk, :], res[:])

    return nc


def _quantize(x, alpha):
    """x [..., 33] f32 -> (hi8, res8) fp8 with alpha*(hi + res/16) ~= x."""
    hi8 = (x / alpha).astype(ml_dtypes.float8_e4m3)
    hi = hi8.astype(np.float32) * alpha
    res8 = ((x - hi) * (RES_SCALE / alpha)).astype(ml_dtypes.float8_e4m3)
    return hi8, res8


def _prep_img(img_core):
    """img_core [HW, 32] f32 -> [P, NPP//2, 2, 66] fp8 (hi|res), alpha-folded."""
    x = np.concatenate(
        [img_core, np.ones((HW, 1), np.float32)], axis=1
    ).reshape(P, NPP, 33)
    alphas = np.where(
        [_col_kind(n) for n in range(NPP)],
        ALPHA_DVE, ALPHA_ACT,
    ).astype(np.float32)[None, :, None]
    hi8, res8 = _quantize(x, alphas)
    out = np.zeros((P, NPP // 2, 2, MDR), dtype=ml_dtypes.float8_e4m3)
    out[:, :, 0, 0:33] = hi8[:, 0::2, :]
    out[:, :, 1, 0:33] = hi8[:, 1::2, :]
    out[:, :, 0, 33:66] = res8[:, 0::2, :]
    out[:, :, 1, 33:66] = res8[:, 1::2, :]
    return out


def kernel(image_output, slic_output, n_segments=S):
    global LAST_EXEC_NS
    import os

    image_output = np.asarray(image_output, dtype=np.float32)
    slic_output = np.asarray(slic_output, dtype=np.int32)

    imgs = image_output.reshape(B, HW, C)
    slics = slic_output.reshape(B, HW)
    iota = np.broadcast_to(
        np.arange(1, S + 1, dtype=np.float16), (P, S)).copy()
    ident = np.eye(33, dtype=np.float32)

    nc = build_kernel()
    nc.compile()

    in_maps = [
        {
            "img8": _prep_img(imgs[b]),
            "slic": np.ascontiguousarray(slics[b]),
            "iota": iota,
            "ident": ident,
        }
        for b in range(B)
    ]

    trace = os.environ.get("KERNEL_TRACE", "0") == "1"
    res = run_bass_kernel_spmd(
        nc, in_maps, core_ids=list(range(N_CORES)), trace=trace
    )
    LAST_EXEC_NS = res.exec_time_ns

    outs = [np.asarray(res.results[b]["out"], dtype=np.float32)
            for b in range(B)]
    return np.stack(outs, axis=1)  # [S, B, C]
